# revision 26
# baseline (speedup 1.0000x reference)
"""MimicAcquisition (double resample: nearest-at-acquisition-res then trilinear
back) as three separable banded-matrix contractions on the PE engine, in bf16.

out[i,j,k] = sum_{a,b,c} Ax[a,i] * Ay[b,j] * Az[c,k] * vol[a,b,c]

where A_d = (trilinear upsample) @ (nearest resample) along axis d is a
192x192 matrix with <=2 nonzeros per row, built on host in float32 arithmetic
that mirrors the reference exactly, then cast to bf16.  Each of the 8 cores
handles one (batch, x'-half, y'-half) octant: host slices a 112x112x192
source band slab (bf16), the device does three matmul passes:

  pass Y: t2[z,j,x]  = sum_y slab[y,x,z] * Ay[y,j]     (rotates z onto partitions)
  pass Z: t3[x,j,k]  = sum_z t2[z,j,x]  * Az[z,k]      (rotates x onto partitions)
  pass X: out[i,j,k] = sum_x Ax[x,i]    * t3[x,j,k]    (terminal, Ax stationary)

bf16 matmuls run 4x the fp32 row rate and lower to a single HW matmul
(fp32 lowers to 2).  t2 stores x contiguously ([z; j, x]) so pass-Z weight
loads are contiguous 128-column LDWEIGHTS (FWL-eligible).  Az is band-limited
(nonzero column ranges asserted host-side) to cut pass-Z matmul rows.
"""

import sys

if "/opt/trn_rl_repo" not in sys.path:
    sys.path.insert(0, "/opt/trn_rl_repo")

import numpy as np

IN = 192          # input extent per axis
RES = 192         # resample (output) extent per axis
H = 112           # padded source-band rows for the sharded axes (x, y)
XP = H            # x extent in t2/t3 (no pad)
OH = 96           # output half extent for sharded axes
Z = 192           # z extent (unsharded)
# Pass Z is split into two disjoint-column matmuls: columns [0, C0) only read
# source rows [0, 128), columns [C0, 192) only read rows [Z1LO, 192).  The two
# z-chunks overlap in [Z1LO, 128) so neither matmul needs accumulation fix-ups.
C0 = 122
Z1LO = 116
Z1N = Z - Z1LO    # 76 partitions in the second z-chunk

_CACHE = {}

LAST_RESULTS = None


# ----------------------------------------------------------------------------
# Host-side table construction (mirrors reference.py float32 arithmetic)
# ----------------------------------------------------------------------------

def _axis_matrix(r):
    """A[src, dst] for one axis given subsample resolution r (float32)."""
    f32 = np.float32
    d = (f32(IN) * f32(1.0) / f32(r)).astype(np.int32)  # down_shape (trunc)
    dz = f32(d) / f32(IN)                               # down_zoom
    uz = f32(RES) / f32(d)                              # up_zoom
    maxl = f32(IN - 1)

    # pass 2 (trilinear) locations for output index i, in mid coordinates
    i = np.arange(RES, dtype=np.float32)
    loc = np.clip(i / uz, f32(0.0), maxl)
    loc0 = np.floor(loc)
    f0 = np.clip(loc0, f32(0.0), maxl)
    f1 = np.clip(loc0 + f32(1.0), f32(0.0), maxl)
    w0 = (f1 - loc).astype(np.float32)      # weight for floor corner
    w1 = (f32(1.0) - w0).astype(np.float32)
    i0 = f0.astype(np.int32)
    i1 = f1.astype(np.int32)

    # pass 1 (nearest) map applied to mid index j
    j = np.arange(IN, dtype=np.float32)
    dl = np.clip(j / dz, f32(0.0), f32(IN))
    g = np.clip(np.round(dl), f32(0.0), maxl).astype(np.int32)

    A = np.zeros((IN, RES), dtype=np.float32)
    cols = np.arange(RES)
    A[g[i0], cols] += w0
    A[g[i1], cols] += w1
    return A


def _band_slice(A, lo, n):
    """Slice rows of A restricted to dst columns [lo, lo+n) into an H-row band.

    Returns (S0, A[S0:S0+H, lo:lo+n]) with all nonzero rows inside the band.
    """
    cols = A[:, lo:lo + n]
    rows = np.nonzero(np.any(cols != 0.0, axis=1))[0]
    rmin, rmax = int(rows[0]), int(rows[-1])
    assert rmax - rmin + 1 <= H, f"band too wide: {rmax - rmin + 1}"
    S0 = min(rmin, IN - H)
    assert rmax < S0 + H
    return S0, np.ascontiguousarray(cols[S0:S0 + H])


# ----------------------------------------------------------------------------
# Device kernel (built once per process)
# ----------------------------------------------------------------------------

def _build(bench_iters=0):
    key = ("nc", bench_iters)
    if key in _CACHE:
        return _CACHE[key]

    import contextlib

    import concourse.mybir as mybir
    from concourse import bacc, tile

    bf16 = mybir.dt.bfloat16
    nc = bacc.Bacc("TRN2", debug=False)

    slab_d = nc.dram_tensor("slab", (H, H, Z), bf16, kind="ExternalInput")
    ax_d = nc.dram_tensor("ax", (XP, OH), bf16, kind="ExternalInput")
    ay_d = nc.dram_tensor("ay", (H, OH), bf16, kind="ExternalInput")
    az0_d = nc.dram_tensor("az0", (128, C0), bf16, kind="ExternalInput")
    az1_d = nc.dram_tensor("az1", (Z1N, RES - C0), bf16, kind="ExternalInput")
    out_d = nc.dram_tensor("out", (OH, OH, Z), bf16, kind="ExternalOutput")

    # slab x-chunk DMA sizes: small leading chunk so pass Y starts early;
    # chunks alternate between the two HWDGE rings (sync / scalar) so one
    # chunk's completion latency hides under the next chunk's transfer
    CHUNKS = [4, 8, 12, 24, 32, 32]
    assert sum(CHUNKS) == H
    XG = 4             # x per psum evac group in pass Y
    JB = 8             # y' per t3 block
    NB = OH // JB      # number of t3 blocks (12)

    with tile.TileContext(nc) as tc:
        loop_cm = (
            tc.For_i(0, bench_iters, 1) if bench_iters
            else contextlib.nullcontext()
        )
        with (
            loop_cm,
            tc.tile_pool(name="consts", bufs=1) as consts,
            tc.tile_pool(name="slab", bufs=1) as slab_pool,
            tc.tile_pool(name="t2", bufs=1) as t2_pool,
            tc.tile_pool(name="t3", bufs=2) as t3_pool,
            tc.tile_pool(name="stage", bufs=3) as stage_pool,
        ):
            ay_t = consts.tile([H, OH], bf16, tag="ay")
            az0_t = consts.tile([128, C0], bf16, tag="az0")
            az1_t = consts.tile([Z1N, RES - C0], bf16, tag="az1")
            ax_t = consts.tile([XP, OH], bf16, tag="ax")

            # slab chunk 0 and ay unblock pass Y; issue their loads first
            slabs = []
            x0s = []
            x0 = 0
            for ci, cw in enumerate(CHUNKS):
                s = slab_pool.tile([H, cw, Z], bf16, tag=f"s{ci}")
                slabs.append(s)
                x0s.append(x0)
                x0 += cw
            nc.sync.dma_start(slabs[0][:], slab_d[:, 0:CHUNKS[0], :])
            nc.scalar.dma_start(slabs[1][:], slab_d[:, x0s[1]:x0s[1] + CHUNKS[1], :])
            nc.sync.dma_start(ay_t[:], ay_d[:])
            nc.scalar.dma_start(az0_t[:], az0_d[:])
            nc.scalar.dma_start(az1_t[:], az1_d[:])
            nc.scalar.dma_start(ax_t[:], ax_d[:])
            for ci in range(2, len(CHUNKS)):
                eng = nc.sync if ci % 2 == 0 else nc.scalar
                eng.dma_start(
                    slabs[ci][:], slab_d[:, x0s[ci]:x0s[ci] + CHUNKS[ci], :]
                )

            # t2 stores x contiguously so pass-Z LDWEIGHTS are contiguous.
            t2a = t2_pool.tile([128, OH, XP], bf16, tag="t2a")
            t2b = t2_pool.tile([Z1N, OH, XP], bf16, tag="t2b")

            def chunk_of(x):
                for ci, lo in enumerate(x0s):
                    if lo <= x < lo + CHUNKS[ci]:
                        return ci, x - lo
                raise AssertionError(x)

            # ---- pass Y: t2[z, j, x] = sum_y slab[y, x, z] * Ay[y, j] ----
            with (
                tc.tile_pool(name="psumw", bufs=1, space="PSUM") as psumw,
                tc.tile_pool(name="psum1", bufs=3, space="PSUM") as psum1,
            ):
                # PE warm-up: tiny matmuls on a zeroed scratch while the slab
                # DMA is in flight, so the HAM clock gate is at 8/8 (and the
                # PE pipeline hot) when the first real matmul issues.
                warm = consts.tile([1, 512], bf16, tag="warm")
                nc.gpsimd.memset(warm[:], 0.0)
                psw = psumw.tile([128, 512], mybir.dt.float32, tag="psw")
                for _ in range(30):
                    nc.tensor.matmul(psw[:], warm[:, 0:128], warm[:])

                for xg in range(H // XG):
                    psA = psum1.tile([128, XG, OH], mybir.dt.float32, tag="psA")
                    psB = psum1.tile([Z1N, XG, OH], mybir.dt.float32, tag="psB")
                    for xi in range(XG):
                        x = xg * XG + xi
                        ci, xl = chunk_of(x)
                        s = slabs[ci]
                        nc.tensor.matmul(psA[:, xi, :], s[:, xl, 0:128], ay_t[:])
                        nc.tensor.matmul(psB[:, xi, :], s[:, xl, Z1LO:Z], ay_t[:])
                    lo = xg * XG
                    dstA = t2a[:, :, lo:lo + XG]
                    dstB = t2b[:, :, lo:lo + XG]
                    srcA = psA[:].transpose([0, 2, 1])
                    srcB = psB[:].transpose([0, 2, 1])
                    if xg % 2 == 0:
                        nc.scalar.copy(dstA, srcA)
                        nc.vector.tensor_copy(dstB, srcB)
                    else:
                        nc.vector.tensor_copy(dstA, srcA)
                        nc.scalar.copy(dstB, srcB)

            # ---- pass Z + pass X, interleaved per 2-y' group ----
            # px(jg) only needs t3[:, 2jg:2jg+2, :], so it chases pz's
            # evacuation group-by-group instead of waiting for the whole block.
            with tc.tile_pool(name="psum2", bufs=3, space="PSUM") as psum2:
                for jb in range(NB):
                    t3 = t3_pool.tile([XP, JB, Z], bf16, tag="t3")
                    stage = stage_pool.tile([OH, JB, Z], bf16, tag="st")
                    for jg in range(JB // 2):
                        pz = psum2.tile([XP, 2, RES], mybir.dt.float32, tag="pz")
                        for ji in range(2):
                            j = jb * JB + jg * 2 + ji
                            nc.tensor.matmul(pz[:, ji, 0:C0], t2a[:, j, :], az0_t[:])
                            nc.tensor.matmul(pz[:, ji, C0:RES], t2b[:, j, :], az1_t[:])
                        t3d = t3[:, jg * 2:jg * 2 + 2, :]
                        px = psum2.tile([OH, 2, Z], mybir.dt.float32, tag="px")
                        std = stage[:, jg * 2:jg * 2 + 2, :]
                        if jg % 2 == 0:
                            nc.scalar.copy(t3d, pz[:])
                            nc.tensor.matmul(px[:], ax_t[:], t3d)
                            nc.vector.tensor_copy(std, px[:])
                        else:
                            nc.vector.tensor_copy(t3d, pz[:])
                            nc.tensor.matmul(px[:], ax_t[:], t3d)
                            nc.scalar.copy(std, px[:])
                    j0 = jb * JB
                    # stores alternate between the sync and gpsimd DGE queues
                    # so issue latency doesn't serialize on one engine; the
                    # final block goes out as two half stores so the last
                    # HBM write receipt covers half the bytes
                    if jb == NB - 1:
                        qj = JB // 4
                        for q in range(4):
                            eng = nc.gpsimd if q % 2 == 0 else nc.sync
                            eng.dma_start(
                                out_d[:, j0 + q * qj:j0 + (q + 1) * qj, :],
                                stage[:, q * qj:(q + 1) * qj, :],
                            )
                    elif jb % 2 == 0:
                        nc.gpsimd.dma_start(out_d[:, j0:j0 + JB, :], stage[:])
                    else:
                        nc.sync.dma_start(out_d[:, j0:j0 + JB, :], stage[:])

    nc.compile()
    _CACHE[key] = nc
    return nc


# ----------------------------------------------------------------------------
# Host wrapper
# ----------------------------------------------------------------------------

def _in_maps(vol, sub):
    import ml_dtypes

    bf16 = ml_dtypes.bfloat16
    maps = []
    spans = []
    tabs = {}
    for core in range(8):
        b = core >> 2
        ix = (core >> 1) & 1
        iy = core & 1
        if b not in tabs:
            tabs[b] = tuple(_axis_matrix(sub[b, d]) for d in range(3))
        Ax, Ay, Az = tabs[b]
        SX, axs = _band_slice(Ax, ix * OH, OH)
        SY, ays = _band_slice(Ay, iy * OH, OH)
        axp = np.zeros((XP, OH), dtype=np.float32)
        axp[:H] = axs
        # disjoint-column Az blocks; assert the band bounds hold
        assert not Az[128:, :C0].any(), "az0 band bound violated"
        assert not Az[:Z1LO, C0:].any(), "az1 band bound violated"
        az0 = np.ascontiguousarray(Az[0:128, 0:C0])
        az1 = np.ascontiguousarray(Az[Z1LO:Z, C0:RES])
        slab = np.ascontiguousarray(
            vol[b, SX:SX + H, SY:SY + H, :, 0].transpose(1, 0, 2)
        )
        maps.append({
            "slab": slab.astype(bf16),
            "ax": axp.astype(bf16),
            "ay": ays.astype(bf16),
            "az0": az0.astype(bf16),
            "az1": az1.astype(bf16),
        })
        spans.append((b, ix * OH, iy * OH))
    return maps, spans


def kernel(vol, subsample_res):
    global LAST_RESULTS
    from concourse import bass_utils

    vol = np.asarray(vol, dtype=np.float32)
    sub = np.asarray(subsample_res, dtype=np.float32)
    nc = _build()
    maps, spans = _in_maps(vol, sub)
    res = bass_utils.run_bass_kernel_spmd(nc, maps, core_ids=list(range(8)))
    LAST_RESULTS = res
    out = np.empty((2, RES, RES, RES, 1), dtype=np.float32)
    for core, (b, x0, y0) in enumerate(spans):
        out[b, x0:x0 + OH, y0:y0 + OH, :, 0] = np.asarray(
            res.results[core]["out"], dtype=np.float32
        )
    return out
